# revision 29
# baseline (speedup 1.0000x reference)
"""Trainium2 Bass kernel for nn_L2MLoRA (fused linear + routed LoRA).

Math (per batch element b, with e = idx[b,0]):
    y[b] = x[b] @ W.T + bias + SCALE * (x[b] @ A_pool[e]) @ B_pool[e]

Strategy: data-parallel over batch B=8 -> one batch element per NeuronCore.
The expert gather (A_pool[e], B_pool[e]) happens on host, so each core gets
exactly one [DIM, RANK] / [RANK, DIM] expert pair. Because there is a single
expert per core, the LoRA term is folded into the base weight ON DEVICE once
at setup:

    W'stat[d, o] = Wstat[d, o] + SCALE * sum_r A[d, r] * B[r, o]

(16 rank-8 matmuls + 64 vector adds, outside the steady-state loop), after
which every iteration is a pure GEMM + bias:

    yT[o, t] = sum_d W'[o, d] * xT[d, t] + bias[o]

Everything is computed in the transposed domain (yT = W' @ xT) so matmul
operands already have the contraction dim on partitions and no on-device
transposes are needed. Inputs/outputs move in bf16 (PE rate is identical to
fp32r, HBM traffic halves: 4MB x in + 2MB W + 4MB y out per core); PSUM
accumulation stays fp32 and the host converts y back to fp32.
"""

import numpy as np

import concourse.bass as bass
import concourse.tile as tile
from concourse import bacc, mybir
from concourse.bass_utils import run_bass_kernel_spmd

B, N, DIM, POOL, RANK = 8, 2048, 1024, 64, 8
SCALE = 2.0
NCORES = 8
P = 128          # partitions / k-tile height / o-chunk width
TW = 512         # token-chunk width (PSUM bank = 512 fp32 free elems)
KT = DIM // P    # 8 k-tiles over the contraction dim
OT = DIM // P    # 8 output chunks
TT = N // TW     # 4 token chunks
KP = KT * P      # 1024
UNROLL = 8       # module iterations per For_i body (amortizes back-edge cost)
F32 = mybir.dt.float32
BF16 = mybir.dt.bfloat16


def build_program(n_iter: int = 1, probe: str = "full"):
    """Build the single-core Tile program (same program runs SPMD on 8 cores).

    n_iter > 1 wraps the body in a For_i loop for benchmarking.
    probe: "full" | "nodma" (x resident, no stores) | "dmaonly" (no matmuls).
    """
    nc = bacc.Bacc("TRN2", target_bir_lowering=False, debug=False,
                   num_devices=NCORES)

    # x:  xt[t, p, k*TW + j] = x[t*TW + j, k*128 + p]   (one 1MB DMA per chunk)
    x_d = nc.dram_tensor("xt", [TT, P, KT * TW], BF16, kind="ExternalInput")
    # W:  wt[p, o*KP + k*128 + c] = W[o*128 + c, k*128 + p]  (stationary layout)
    w_d = nc.dram_tensor("wt", [P, OT * KP], BF16, kind="ExternalInput")
    # A^T pre-transposed:  at[r, d] = A[d, r]
    at_d = nc.dram_tensor("at", [RANK, DIM], BF16, kind="ExternalInput")
    # SCALE * B:  bp[r, o*128 + c] = SCALE * B[r, o*128 + c]
    bp_d = nc.dram_tensor("bp", [RANK, DIM], BF16, kind="ExternalInput")
    bias_d = nc.dram_tensor("bias", [P, OT], F32, kind="ExternalInput")
    # y: y[t, c, o*TW + j] = y_full[t*TW + j, o*128 + c]
    y_d = nc.dram_tensor("y", [TT, P, OT * TW], BF16, kind="ExternalOutput")

    with tile.TileContext(nc) as tc:
        with (
            tc.tile_pool(name="cpool", bufs=1) as cpool,
            tc.tile_pool(name="opool", bufs=6) as opool,
            tc.tile_pool(name="psy", bufs=2, space="PSUM") as psy_pool,
        ):
            def load_full(dst):
                for t in range(TT):
                    nc.sync.dma_start(
                        dst[:, t * KT * TW:(t + 1) * KT * TW], x_d.ap()[t]
                    )

            # Constants: loaded once, persist across benchmark iterations.
            at_sb = cpool.tile([RANK, DIM], BF16, tag="at")
            nc.sync.dma_start(at_sb[:], at_d.ap()[:])
            bp_sb = cpool.tile([RANK, DIM], BF16, tag="bp")
            nc.sync.dma_start(bp_sb[:], bp_d.ap()[:])
            bias_sb = cpool.tile([P, OT], F32, tag="bias")
            nc.sync.dma_start(bias_sb[:], bias_d.ap()[:])
            # Persistent full-x ping-pong pair (32KB/partition each).
            xa = cpool.tile([P, TT * KT * TW], BF16, tag="xa")
            load_full(xa)
            xb = (cpool.tile([P, TT * KT * TW], BF16, name="xb", tag="xb")
                  if n_iter > 1 and probe != "nodma" else None)
            w_sb = cpool.tile([P, OT * KP], BF16, tag="w")
            nc.sync.dma_start(w_sb[:], w_d.ap()[:])

            # One-time fold: W' = W + SCALE * (A @ B) in the stationary layout.
            # delta_stat[d, o] = sum_r at[r, d] * bp[r, o], done per k-chunk of
            # d (128 partitions) x half of o (512 free).
            for k in range(KT):
                for h in range(2):
                    ps = psy_pool.tile([P, TW], F32, name="ps0", tag="ps0")
                    nc.tensor.matmul(
                        ps[:],
                        at_sb[:, k * P:(k + 1) * P],
                        bp_sb[:, h * TW:(h + 1) * TW],
                        start=True, stop=True,
                    )
                    for oo in range(4):
                        o = h * 4 + oo
                        w_slice = w_sb[:, o * KP + k * P: o * KP + (k + 1) * P]
                        nc.vector.tensor_add(
                            w_slice, w_slice, ps[:, oo * P:(oo + 1) * P]
                        )

            def body(xt, other=None):
                """One module iteration, k-major with the 4 t-chunks
                interleaved across 4 PSUM banks so each stationary weight
                tile is loaded once and streamed 4x (ldweights elided on the
                3 repeats).  `xt` is a full-x persistent tile; `other` (if
                given) is refilled for the next iteration."""
                if other is not None:
                    load_full(other)
                obs = [opool.tile([P, OT, TW], BF16, name="ob", tag="ob")
                       for _ in range(TT)]
                if probe != "dmaonly":
                    for o in range(OT):
                        pss = [psy_pool.tile([P, TW], F32, name=f"ps{t}", tag=f"ps{t}")
                               for t in range(TT)]
                        for k in range(KT):
                            w_slice = w_sb[:, o * KP + k * P: o * KP + (k + 1) * P]
                            # Standalone LDWEIGHTS: the PE reorder window pulls
                            # it ahead into the background weight buffer while
                            # the previous group's matmuls stream, so no matmul
                            # serializes with its own weight load.
                            nc.tensor.ldweights(w_slice)
                            for t in range(TT):
                                m = nc.tensor.matmul(
                                    pss[t][:],
                                    w_slice,
                                    xt[:, (t * KT + k) * TW:(t * KT + k + 1) * TW],
                                    start=(k == 0), stop=(k == KT - 1),
                                )
                                m.ins.ldweights = False
                        for t in range(TT):
                            nc.scalar.activation(
                                obs[t][:, o, :], pss[t][:],
                                mybir.ActivationFunctionType.Identity,
                                bias=bias_sb[:, o:o + 1], scale=1.0,
                            )
                if probe != "nodma":
                    # stores on the ACT HWDGE ring: ready exactly when the
                    # activations finish, and they never block SP's loads.
                    for t in range(TT):
                        nc.scalar.dma_start(y_d.ap()[t], obs[t][:])

            if n_iter == 1:
                body(xa)
            else:
                # The For_i back-edge costs ~6-9us of PE idle (engine drain +
                # staggered semaphore resets) and re-throttles the PE clock
                # gate.  Unrolling UNROLL module iterations per loop body
                # amortizes that cost, and the persistent xa/xb ping-pong
                # carries x across the back-edge so the first matmuls of a
                # pass never wait on a post-barrier DMA.
                assert n_iter % UNROLL == 0 and UNROLL % 2 == 0, (n_iter, UNROLL)
                with tc.For_i(0, n_iter // UNROLL, 1,
                              hint_engines=tuple(mybir.ALL_ENGINES),
                              staggered_reset=True):
                    for u in range(UNROLL):
                        cur, nxt = (xa, xb) if u % 2 == 0 else (xb, xa)
                        body(cur, other=(nxt if probe != "nodma" else None))

    nc.compile()
    return nc


def make_in_maps(x, idx, weight, bias, A_pool, B_pool):
    """Host-side shard + relayout. Returns per-core input dicts."""
    bf16 = mybir.dt.np(BF16)
    x = np.asarray(x, dtype=np.float32)
    idx = np.asarray(idx)
    weight = np.asarray(weight, dtype=np.float32)
    bias = np.asarray(bias, dtype=np.float32)
    A_pool = np.asarray(A_pool, dtype=np.float32)
    B_pool = np.asarray(B_pool, dtype=np.float32)

    # W[o, d] -> wt[p(=d within k), o*KP + k*128 + c(=o within chunk)]
    wt = np.ascontiguousarray(
        weight.reshape(OT, P, KT, P).transpose(3, 0, 2, 1).reshape(P, OT * KP)
    ).astype(bf16)
    bias_t = np.ascontiguousarray(bias.reshape(OT, P).T)  # [p, o_chunk]

    sel = idx.reshape(B).astype(np.int64)
    in_maps = []
    for c in range(NCORES):
        # x[n, d] -> xt[t, p, k*TW + j]
        xt = np.ascontiguousarray(
            x[c].reshape(TT, TW, KT, P).transpose(0, 3, 2, 1).reshape(TT, P, KT * TW)
        ).astype(bf16)
        at = np.ascontiguousarray(A_pool[sel[c]].T).astype(bf16)   # [RANK, DIM]
        bp = (SCALE * B_pool[sel[c]]).astype(bf16)                 # [RANK, DIM]
        in_maps.append({"xt": xt, "wt": wt, "at": at, "bp": bp, "bias": bias_t})
    return in_maps


def assemble_output(results):
    """Per-core y blocks [TT, P, OT*TW] -> full [B, N, DIM] fp32 output."""
    out = np.empty((B, N, DIM), dtype=np.float32)
    for c in range(NCORES):
        yb = np.asarray(results[c]["y"]).astype(np.float32)
        # yb[t, c_, o*TW + j] = y[c, t*TW + j, o*128 + c_]
        out[c] = yb.reshape(TT, P, OT, TW).transpose(0, 3, 2, 1).reshape(N, DIM)
    return out


_PROGRAM_CACHE = {}


def _get_program(n_iter: int = 1):
    if n_iter not in _PROGRAM_CACHE:
        _PROGRAM_CACHE[n_iter] = build_program(n_iter)
    return _PROGRAM_CACHE[n_iter]


def kernel(x, idx, frozen_mask, weight, bias, A_pool, B_pool):
    # frozen_mask only affects gradients (stop_gradient); forward is identical.
    nc = _get_program(1)
    in_maps = make_in_maps(x, idx, weight, bias, A_pool, B_pool)
    res = run_bass_kernel_spmd(nc, in_maps, list(range(NCORES)))
    return assemble_output(res.results)


# revision 30
# speedup vs baseline: 1.0438x; 1.0438x over previous
"""Trainium2 Bass kernel for nn_L2MLoRA (fused linear + routed LoRA).

Math (per batch element b, with e = idx[b,0]):
    y[b] = x[b] @ W.T + bias + SCALE * (x[b] @ A_pool[e]) @ B_pool[e]

Strategy: data-parallel over batch B=8 -> one batch element per NeuronCore.
The expert gather (A_pool[e], B_pool[e]) happens on host, so each core gets
exactly one [DIM, RANK] / [RANK, DIM] expert pair. Because there is a single
expert per core, the LoRA term is folded into the base weight ON DEVICE once
at setup:

    W'stat[d, o] = Wstat[d, o] + SCALE * sum_r A[d, r] * B[r, o]

(16 rank-8 matmuls + 64 vector adds, outside the steady-state loop), after
which every iteration is a pure GEMM + bias:

    yT[o, t] = sum_d W'[o, d] * xT[d, t] + bias[o]

Everything is computed in the transposed domain (yT = W' @ xT) so matmul
operands already have the contraction dim on partitions and no on-device
transposes are needed. Inputs/outputs move in bf16 (PE rate is identical to
fp32r, HBM traffic halves: 4MB x in + 2MB W + 4MB y out per core); PSUM
accumulation stays fp32 and the host converts y back to fp32.
"""

import numpy as np

import concourse.bass as bass
import concourse.tile as tile
from concourse import bacc, mybir
from concourse.bass_utils import run_bass_kernel_spmd

B, N, DIM, POOL, RANK = 8, 2048, 1024, 64, 8
SCALE = 2.0
NCORES = 8
P = 128          # partitions / k-tile height / o-chunk width
TW = 512         # token-chunk width (PSUM bank = 512 fp32 free elems)
KT = DIM // P    # 8 k-tiles over the contraction dim
OT = DIM // P    # 8 output chunks
TT = N // TW     # 4 token chunks
KP = KT * P      # 1024
UNROLL = 8       # module iterations per For_i body (amortizes back-edge cost)
F32 = mybir.dt.float32
BF16 = mybir.dt.bfloat16


def build_program(n_iter: int = 1, probe: str = "full"):
    """Build the single-core Tile program (same program runs SPMD on 8 cores).

    n_iter > 1 wraps the body in a For_i loop for benchmarking.
    probe: "full" | "nodma" (x resident, no stores) | "dmaonly" (no matmuls).
    """
    nc = bacc.Bacc("TRN2", target_bir_lowering=False, debug=False,
                   num_devices=NCORES)

    # x:  xt[t, p, k*TW + j] = x[t*TW + j, k*128 + p]   (one 1MB DMA per chunk)
    x_d = nc.dram_tensor("xt", [TT, P, KT * TW], BF16, kind="ExternalInput")
    # W:  wt[p, o*KP + k*128 + c] = W[o*128 + c, k*128 + p]  (stationary layout)
    w_d = nc.dram_tensor("wt", [P, OT * KP], BF16, kind="ExternalInput")
    # A^T pre-transposed:  at[r, d] = A[d, r]
    at_d = nc.dram_tensor("at", [RANK, DIM], BF16, kind="ExternalInput")
    # SCALE * B:  bp[r, o*128 + c] = SCALE * B[r, o*128 + c]
    bp_d = nc.dram_tensor("bp", [RANK, DIM], BF16, kind="ExternalInput")
    bias_d = nc.dram_tensor("bias", [P, OT], F32, kind="ExternalInput")
    # y: y[t, c, o*TW + j] = y_full[t*TW + j, o*128 + c]
    y_d = nc.dram_tensor("y", [TT, P, OT * TW], BF16, kind="ExternalOutput")

    with tile.TileContext(nc) as tc:
        with (
            tc.tile_pool(name="cpool", bufs=1) as cpool,
            tc.tile_pool(name="opool", bufs=6) as opool,
            tc.tile_pool(name="psy", bufs=2, space="PSUM") as psy_pool,
        ):
            def load_full(dst):
                for t in range(TT):
                    nc.sync.dma_start(
                        dst[:, t * KT * TW:(t + 1) * KT * TW], x_d.ap()[t]
                    )

            # Constants: loaded once, persist across benchmark iterations.
            at_sb = cpool.tile([RANK, DIM], BF16, tag="at")
            nc.sync.dma_start(at_sb[:], at_d.ap()[:])
            bp_sb = cpool.tile([RANK, DIM], BF16, tag="bp")
            nc.sync.dma_start(bp_sb[:], bp_d.ap()[:])
            bias_sb = cpool.tile([P, OT], F32, tag="bias")
            nc.sync.dma_start(bias_sb[:], bias_d.ap()[:])
            # Persistent full-x ping-pong pair (32KB/partition each).
            xa = cpool.tile([P, TT * KT * TW], BF16, tag="xa")
            load_full(xa)
            xb = (cpool.tile([P, TT * KT * TW], BF16, name="xb", tag="xb")
                  if n_iter > 1 and probe != "nodma" else None)
            w_sb = cpool.tile([P, OT * KP], BF16, tag="w")
            nc.sync.dma_start(w_sb[:], w_d.ap()[:])

            # One-time fold: W' = W + SCALE * (A @ B) in the stationary layout.
            # delta_stat[d, o] = sum_r at[r, d] * bp[r, o], done per k-chunk of
            # d (128 partitions) x half of o (512 free).
            for k in range(KT):
                for h in range(2):
                    ps = psy_pool.tile([P, TW], F32, name="ps0", tag="ps0")
                    nc.tensor.matmul(
                        ps[:],
                        at_sb[:, k * P:(k + 1) * P],
                        bp_sb[:, h * TW:(h + 1) * TW],
                        start=True, stop=True,
                    )
                    for oo in range(4):
                        o = h * 4 + oo
                        w_slice = w_sb[:, o * KP + k * P: o * KP + (k + 1) * P]
                        nc.vector.tensor_add(
                            w_slice, w_slice, ps[:, oo * P:(oo + 1) * P]
                        )

            def body(xt, other=None):
                """One module iteration, k-major with the 4 t-chunks
                interleaved across 4 PSUM banks so each stationary weight
                tile is loaded once and streamed 4x (ldweights elided on the
                3 repeats).  `xt` is a full-x persistent tile; `other` (if
                given) is refilled for the next iteration."""
                if other is not None:
                    load_full(other)
                obs = [opool.tile([P, OT, TW], BF16, name="ob", tag="ob")
                       for _ in range(TT)]
                if probe != "dmaonly":
                    for o in range(OT):
                        pss = [psy_pool.tile([P, TW], F32, name=f"ps{t}", tag=f"ps{t}")
                               for t in range(TT)]
                        for k in range(KT):
                            for t in range(TT):
                                m = nc.tensor.matmul(
                                    pss[t][:],
                                    w_sb[:, o * KP + k * P: o * KP + (k + 1) * P],
                                    xt[:, (t * KT + k) * TW:(t * KT + k + 1) * TW],
                                    start=(k == 0), stop=(k == KT - 1),
                                )
                                if t > 0:
                                    m.ins.ldweights = False
                        for t in range(TT):
                            nc.scalar.activation(
                                obs[t][:, o, :], pss[t][:],
                                mybir.ActivationFunctionType.Identity,
                                bias=bias_sb[:, o:o + 1], scale=1.0,
                            )
                if probe != "nodma":
                    # stores on the ACT HWDGE ring: ready exactly when the
                    # activations finish, and they never block SP's loads.
                    for t in range(TT):
                        nc.scalar.dma_start(y_d.ap()[t], obs[t][:])

            if n_iter == 1:
                body(xa)
            else:
                # The For_i back-edge costs ~6-9us of PE idle (engine drain +
                # staggered semaphore resets) and re-throttles the PE clock
                # gate.  Unrolling UNROLL module iterations per loop body
                # amortizes that cost, and the persistent xa/xb ping-pong
                # carries x across the back-edge so the first matmuls of a
                # pass never wait on a post-barrier DMA.
                assert n_iter % UNROLL == 0 and UNROLL % 2 == 0, (n_iter, UNROLL)
                with tc.For_i(0, n_iter // UNROLL, 1,
                              hint_engines=tuple(mybir.ALL_ENGINES),
                              staggered_reset=True):
                    for u in range(UNROLL):
                        cur, nxt = (xa, xb) if u % 2 == 0 else (xb, xa)
                        body(cur, other=(nxt if probe != "nodma" else None))

    nc.compile()
    return nc


def make_in_maps(x, idx, weight, bias, A_pool, B_pool):
    """Host-side shard + relayout. Returns per-core input dicts."""
    bf16 = mybir.dt.np(BF16)
    x = np.asarray(x, dtype=np.float32)
    idx = np.asarray(idx)
    weight = np.asarray(weight, dtype=np.float32)
    bias = np.asarray(bias, dtype=np.float32)
    A_pool = np.asarray(A_pool, dtype=np.float32)
    B_pool = np.asarray(B_pool, dtype=np.float32)

    # W[o, d] -> wt[p(=d within k), o*KP + k*128 + c(=o within chunk)]
    wt = np.ascontiguousarray(
        weight.reshape(OT, P, KT, P).transpose(3, 0, 2, 1).reshape(P, OT * KP)
    ).astype(bf16)
    bias_t = np.ascontiguousarray(bias.reshape(OT, P).T)  # [p, o_chunk]

    sel = idx.reshape(B).astype(np.int64)
    in_maps = []
    for c in range(NCORES):
        # x[n, d] -> xt[t, p, k*TW + j]
        xt = np.ascontiguousarray(
            x[c].reshape(TT, TW, KT, P).transpose(0, 3, 2, 1).reshape(TT, P, KT * TW)
        ).astype(bf16)
        at = np.ascontiguousarray(A_pool[sel[c]].T).astype(bf16)   # [RANK, DIM]
        bp = (SCALE * B_pool[sel[c]]).astype(bf16)                 # [RANK, DIM]
        in_maps.append({"xt": xt, "wt": wt, "at": at, "bp": bp, "bias": bias_t})
    return in_maps


def assemble_output(results):
    """Per-core y blocks [TT, P, OT*TW] -> full [B, N, DIM] fp32 output."""
    out = np.empty((B, N, DIM), dtype=np.float32)
    for c in range(NCORES):
        yb = np.asarray(results[c]["y"]).astype(np.float32)
        # yb[t, c_, o*TW + j] = y[c, t*TW + j, o*128 + c_]
        out[c] = yb.reshape(TT, P, OT, TW).transpose(0, 3, 2, 1).reshape(N, DIM)
    return out


_PROGRAM_CACHE = {}


def _get_program(n_iter: int = 1):
    if n_iter not in _PROGRAM_CACHE:
        _PROGRAM_CACHE[n_iter] = build_program(n_iter)
    return _PROGRAM_CACHE[n_iter]


def kernel(x, idx, frozen_mask, weight, bias, A_pool, B_pool):
    # frozen_mask only affects gradients (stop_gradient); forward is identical.
    nc = _get_program(1)
    in_maps = make_in_maps(x, idx, weight, bias, A_pool, B_pool)
    res = run_bass_kernel_spmd(nc, in_maps, list(range(NCORES)))
    return assemble_output(res.results)
